# revision 1
# baseline (speedup 1.0000x reference)
"""Fused multi-head attention block (qkv + rmsnorm + rope + sdpa + proj) for
Trainium2, sharded over 8 NeuronCores as batch x head-half (Megatron-style).

Shapes (hardcoded): B=4, N=2048, C=1024, H=16, D=64.
Each core handles one batch and 8 heads (= 512 feature columns).
Host folds qn_w/kn_w into rope tables, sums the two per-batch partial
projection outputs and transposes back to [B, N, C].
"""
import os
import sys

os.environ.setdefault("NEURON_RT_RESET_CORES", "1")
sys.path.insert(0, "/opt/trn_rl_repo")

import ml_dtypes
import numpy as np

import concourse.bass as bass
import concourse.mybir as mybir
import concourse.tile as tile
from concourse import bacc
from concourse.bass_utils import run_bass_kernel_spmd
from concourse.masks import make_identity

dt = mybir.dt
F32 = dt.float32
F32R = dt.float32r
BF16 = dt.bfloat16
AF = mybir.ActivationFunctionType

B, N, C, H, D = 4, 2048, 1024, 16, 64
HL = H // 2          # heads per core = 8
FL = HL * D          # local features = 512
EPS = 1e-6
SCALE = 1.0 / np.sqrt(D)
NCHUNK = N // 128    # 16
KC = C // 128        # 8  (c_in chunks)
IH = 2               # i-halves of 1024 in phase 2

_PROGRAM = None


def _build_program(with_qkv_bias, with_proj_bias, bench_reps=0, qkv_bf16=True):
    XDT = BF16 if qkv_bf16 else F32R
    xlb, nqb, scb, ptb, avb = (5, 6, 10, 4, 2) if qkv_bf16 else (3, 4, 8, 3, 1)
    nc = bacc.Bacc("TRN2", target_bir_lowering=False, debug=False, num_devices=8)

    i_xT = nc.dram_tensor("xT", [NCHUNK, 128, KC, 128], XDT, kind="ExternalInput")
    i_wq = nc.dram_tensor("wq", [C, FL], XDT, kind="ExternalInput")
    i_wk = nc.dram_tensor("wk", [C, FL], XDT, kind="ExternalInput")
    i_wv = nc.dram_tensor("wv", [C, FL], XDT, kind="ExternalInput")
    i_wp = nc.dram_tensor("wp", [FL, C], BF16, kind="ExternalInput")
    if with_qkv_bias:
        i_qkvb = nc.dram_tensor("qkvb", [1, 3 * FL], XDT, kind="ExternalInput")
        i_ones1 = nc.dram_tensor("ones1", [1, 128], XDT, kind="ExternalInput")
    i_raq = nc.dram_tensor("raq", [N, D], BF16, kind="ExternalInput")
    i_rbq = nc.dram_tensor("rbq", [N, D], BF16, kind="ExternalInput")
    i_rak = nc.dram_tensor("rak", [N, D], BF16, kind="ExternalInput")
    i_rbk = nc.dram_tensor("rbk", [N, D], BF16, kind="ExternalInput")
    if with_proj_bias:
        i_pb = nc.dram_tensor("pb", [128, KC], F32, kind="ExternalInput")
    o_FT = nc.dram_tensor("FT", [C, N], F32, kind="ExternalOutput")
    d_rl = nc.dram_tensor("d_rl", [HL, IH, 1024], F32)  # recip-sum bounce

    from contextlib import ExitStack
    with tile.TileContext(nc) as tc:
        with ExitStack() as ctx:
            pp = ctx.enter_context(tc.tile_pool(name="persist", bufs=1))
            wpool = ctx.enter_context(tc.tile_pool(name="wpool", bufs=1))
            xload = ctx.enter_context(tc.tile_pool(name="xload", bufs=xlb))
            scratch = ctx.enter_context(tc.tile_pool(name="scratch", bufs=scb))
            natq = ctx.enter_context(tc.tile_pool(name="natq", bufs=nqb))
            natk = ctx.enter_context(tc.tile_pool(name="natk", bufs=nqb))
            sumsp = ctx.enter_context(tc.tile_pool(name="sumsp", bufs=4))
            ptp = ctx.enter_context(tc.tile_pool(name="ptp", bufs=ptb))
            rlp = ctx.enter_context(tc.tile_pool(name="rlp", bufs=2))
            rbp = ctx.enter_context(tc.tile_pool(name="rbp", bufs=2))
            outp = ctx.enter_context(tc.tile_pool(name="outp", bufs=2))
            # one PSUM pool, 4 tags x 2 banks (8 banks total), shared by all
            # phases: scA/scB + avA/avB
            psp = ctx.enter_context(tc.tile_pool(name="psp", bufs=1, space="PSUM"))

            # ---- persistent tensors ----
            w_sb = {}
            for nm, src in (("q", i_wq), ("k", i_wk), ("v", i_wv)):
                t = wpool.tile([128, KC, FL], XDT, tag=f"w{nm}")
                nc.sync.dma_start(out=t[:], in_=src[:].rearrange(
                    "(kc c) f -> c kc f", c=128))
                w_sb[nm] = t
            if with_qkv_bias:
                qkvb_sb = wpool.tile([1, 3 * FL], XDT, tag="qkvb")
                nc.sync.dma_start(out=qkvb_sb[:], in_=i_qkvb[:])
                ones1 = wpool.tile([1, 128], XDT, tag="ones1")
                nc.sync.dma_start(out=ones1[:], in_=i_ones1[:])
            if with_proj_bias:
                pb_sb = wpool.tile([128, KC], F32, tag="pb")
                nc.sync.dma_start(out=pb_sb[:], in_=i_pb[:])
            eps_sb = wpool.tile([128, 1], F32, tag="eps")
            nc.vector.memset(eps_sb[:], EPS)
            ident = wpool.tile([128, 128], BF16, tag="ident")
            make_identity(nc, ident[:])

            qT = pp.tile([128, 4, N], BF16, tag="qT")     # [f%128, fc, n]
            kT = pp.tile([128, 4, N], BF16, tag="kT")
            yT = pp.tile([128, 4, N], BF16, tag="yT")
            vball = pp.tile([128, NCHUNK, HL * 96], BF16, tag="vball")
            # per-head 96-col block: [1 | zeros(31) | v(64)].  Even head h
            # slices [96h+32, 96h+160): oT rows 0-63, sums row 64.  Odd head h
            # slices [96h-32, 96h+96): oT rows 64-127, sums row 32.  Unused
            # psum rows collect garbage and are never read.
            vview = vball[:].rearrange("p jc (h c) -> p jc h c", c=96)
            nc.vector.memset(vview[:, :, :, 0:32], 0.0)
            nc.vector.memset(vview[:, :, :, 0:1], 1.0)

            rope_sb = {}
            for nm2, srct in (("raq", i_raq), ("rbq", i_rbq),
                              ("rak", i_rak), ("rbk", i_rbk)):
                rt = wpool.tile([128, NCHUNK, D], BF16, tag=nm2)
                nc.sync.dma_start(out=rt[:], in_=srct[:].rearrange(
                    "(m p) d -> p m d", p=128))
                rope_sb[nm2] = rt

            wp_sb = wpool.tile([128, 4, C], BF16, tag="wp")
            nc.sync.dma_start(out=wp_sb[:], in_=i_wp[:].rearrange(
                "(kc c) o -> c kc o", c=128))

            # ================= Phase 1: qkv + rmsnorm + rope + transpose ====
            # Software-pipelined across n-chunks: S1 (qkv matmuls + psum
            # copies), S2 (rms stats + rope), S3 (PE transposes) are emitted
            # with a skew so each engine's FIFO interleaves iterations.
            loop_ctx = tc.For_i(0, bench_reps, 1) if bench_reps else None
            if loop_ctx is not None:
                ctx.enter_context(loop_ctx)

            nat_t = {}
            ro_t = {}

            def stage1(m):
                nsl = slice(m * 128, (m + 1) * 128)
                xm = xload.tile([128, KC, 128], XDT, tag="xm", name=f"xm{m}")
                nc.sync.dma_start(out=xm[:], in_=i_xT[m])
                nat = {"q": natq.tile([128, FL], BF16, tag="qnat", name=f"qnat{m}"),
                       "k": natk.tile([128, FL], BF16, tag="knat", name=f"knat{m}")}
                nat_t[m] = nat
                for ti, nm in enumerate(("q", "k", "v")):
                    for ft in range(2):
                        fsl = slice(ft * 256, (ft + 1) * 256)
                        psum = psp.tile([128, 256], F32, tag=f"sc{ft}",
                                        name=f"qkvps{m}_{ti}_{ft}")
                        if with_qkv_bias:
                            nc.tensor.matmul(
                                psum[:], ones1[:],
                                qkvb_sb[:, ti * FL + ft * 256:
                                        ti * FL + (ft + 1) * 256],
                                start=True, stop=False)
                        for kc in range(KC):
                            nc.tensor.matmul(psum[:], xm[:, kc, :],
                                             w_sb[nm][:, kc, fsl],
                                             start=(kc == 0 and not with_qkv_bias),
                                             stop=(kc == KC - 1))
                        if nm == "v":
                            # straight into AV layout [n_p, jc=m, head, d]
                            nc.scalar.copy(
                                out=vview[:, m, ft * 4:(ft + 1) * 4, 32:96],
                                in_=psum[:].rearrange("p (h d) -> p h d", h=4))
                        else:
                            nc.scalar.copy(out=nat[nm][:, fsl], in_=psum[:])

            rstd_t = {}

            def stage2a(m):
                nat = nat_t[m]
                sums = sumsp.tile([128, 2 * HL], F32, tag="sums",
                                  name=f"sums{m}")
                for si, nm in enumerate(("q", "k")):
                    sq = scratch.tile([128, FL], BF16, tag="sxt",
                                      name=f"sq_{m}_{si}")
                    nc.gpsimd.tensor_mul(sq[:], nat[nm][:], nat[nm][:])
                    nc.vector.tensor_reduce(
                        sums[:, si * HL:(si + 1) * HL],
                        sq[:].rearrange("p (h d) -> p h d", h=HL),
                        axis=mybir.AxisListType.X, op=mybir.AluOpType.add)
                rstd_t[m] = sums

            def stage2b(m):
                nsl = slice(m * 128, (m + 1) * 128)
                nat = nat_t.pop(m)
                sums = rstd_t.pop(m)
                lns = sumsp.tile([128, 2 * HL], F32, tag="lns", name=f"lns{m}")
                nc.scalar.activation(lns[:], sums[:], AF.Ln,
                                     scale=1.0 / D, bias=eps_sb[:])
                rstd = sumsp.tile([128, 2 * HL], BF16, tag="rstd",
                                  name=f"rstd{m}")
                nc.scalar.activation(rstd[:], lns[:], AF.Exp, scale=-0.5)
                for si, (nm, ra, rb) in enumerate(
                        (("q", "raq", "rbq"), ("k", "rak", "rbk"))):
                    rat = rope_sb[ra][:, m, :]
                    rbt = rope_sb[rb][:, m, :]

                    xv = nat[nm][:].rearrange("p (h d) -> p h d", h=HL)
                    rsview = bass.AP(
                        tensor=rstd.tensor, offset=rstd[:].offset + si * HL,
                        ap=[rstd[:].ap[0], [1, HL], [0, D]])
                    xn = scratch.tile([128, FL], BF16, tag="sxt",
                                      name=f"xn_{m}_{si}")
                    xnv = xn[:].rearrange("p (h d) -> p h d", h=HL)
                    nc.vector.tensor_mul(xnv, xv, rsview)

                    rav = bass.AP(tensor=rat.tensor, offset=rat.offset,
                                  ap=[rat.ap[0], [0, HL], [1, D]])
                    t1 = scratch.tile([128, FL], BF16, tag="sxt",
                                      name=f"t1_{m}_{si}")
                    t1v = t1[:].rearrange("p (h d) -> p h d", h=HL)
                    nc.vector.tensor_mul(t1v, xnv, rav)

                    ro = scratch.tile([128, FL], BF16, tag="ro", bufs=4,
                                      name=f"ro_{m}_{si}")
                    rov = ro[:].rearrange("p (h d) -> p h d", h=HL)
                    rb_lo = bass.AP(tensor=rbt.tensor, offset=rbt.offset,
                                    ap=[rbt.ap[0], [0, HL], [1, 32]])
                    rb_hi = bass.AP(tensor=rbt.tensor, offset=rbt.offset + 32,
                                    ap=[rbt.ap[0], [0, HL], [1, 32]])
                    nc.vector.tensor_mul(rov[:, :, 0:32], xnv[:, :, 32:64], rb_lo)
                    nc.vector.tensor_mul(rov[:, :, 32:64], xnv[:, :, 0:32], rb_hi)
                    nc.gpsimd.tensor_add(ro[:], ro[:], t1[:])
                    ro_t[(m, si)] = ro

            def stage3(m):
                nsl = slice(m * 128, (m + 1) * 128)
                for si, dst in ((0, qT), (1, kT)):
                    ro = ro_t.pop((m, si))
                    for fc in range(4):
                        tp = psp.tile([128, 128], BF16, tag=f"av{'AB'[fc % 2]}",
                                      name=f"trps{m}_{si}_{fc}")
                        nc.tensor.transpose(tp[:], ro[:, fc * 128:(fc + 1) * 128],
                                            ident[:])
                        nc.vector.tensor_copy(out=dst[:, fc, nsl], in_=tp[:])

            for t in range(NCHUNK + 3):
                if t >= 3:
                    stage3(t - 3)
                if 2 <= t < NCHUNK + 2:
                    stage2b(t - 2)
                if 1 <= t < NCHUNK + 1:
                    stage2a(t - 1)
                if t < NCHUNK:
                    stage1(t)

            # ================= Phase 2: attention per head-pair =============
            PHASES = int(os.environ.get("KERNEL_PHASES", "3"))
            for fc in range(4 if PHASES >= 2 else 0):
                for ih in range(IH):
                    isl = slice(ih * 1024, (ih + 1) * 1024)
                    av = {0: psp.tile([128, 1024], F32, tag="avA", name=f"avA{fc}_{ih}"),
                          1: psp.tile([128, 1024], F32, tag="avB", name=f"avB{fc}_{ih}")}
                    # per jc: scores (both halves) -> exps -> AVs of the
                    # previous jc (one-step skew keeps PE off ACT's critical
                    # path; pt bufs=3 covers the extra lifetime)
                    pt_t = {}

                    def do_av(jc):
                        for half in range(2):
                            h = 2 * fc + half
                            vs = 96 * h + 32 if half == 0 else 96 * h - 32
                            pt = pt_t.pop((jc, half))
                            for i2 in range(2):
                                nc.tensor.matmul(
                                    av[half][:, i2 * 512:(i2 + 1) * 512],
                                    vball[:, jc, vs:vs + 128],
                                    pt[:, i2 * 512:(i2 + 1) * 512],
                                    start=(jc == 0), stop=(jc == NCHUNK - 1))

                    for jc in range(NCHUNK):
                        jsl = slice(jc * 128, (jc + 1) * 128)
                        scs = []
                        for half in range(2):
                            po = 64 * half
                            sc = psp.tile([128, 1024], F32, tag=f"sc{half}",
                                          name=f"sc{fc}_{ih}_{jc}_{half}")
                            lhs = kT[po:po + 64, fc, jsl]
                            for i2 in range(2):
                                nc.tensor.matmul(
                                    sc[:, i2 * 512:(i2 + 1) * 512], lhs,
                                    qT[po:po + 64, fc,
                                       ih * 1024 + i2 * 512:
                                       ih * 1024 + (i2 + 1) * 512],
                                    start=True, stop=True)
                            scs.append(sc)
                        if os.environ.get("KERNEL_NOSM"):
                            continue
                        for half in range(2):
                            pt = ptp.tile([128, 1024], BF16, tag=f"pt{half}",
                                          name=f"pt{fc}_{ih}_{jc}_{half}")
                            nc.scalar.activation(pt[:], scs[half][:], AF.Exp,
                                                 scale=float(SCALE))
                            pt_t[(jc, half)] = pt
                        if jc > 0:
                            do_av(jc - 1)
                    if not os.environ.get("KERNEL_NOSM"):
                        do_av(NCHUNK - 1)
                    # normalize: yT[f, n] = av_oT[f, n] * (1 / av_sums[n]).
                    # even head: oT rows 0-63, sums row 64; odd head: oT rows
                    # 64-127, sums row 63 (lane-aligned by vball layout).
                    # copy AV psum -> sbuf right away (frees psum banks for
                    # the next i-half); reciprocal + DRAM-bounce broadcast +
                    # final mult then run entirely off the psum critical path.
                    for half in range(2 if not os.environ.get("KERNEL_NONORM") else 0):
                        po = 64 * half
                        srow = 64 if half == 0 else 32
                        h = 2 * fc + half
                        avs = rlp.tile([128, 1024], F32, tag="avs", bufs=avb,
                                       name=f"avs{fc}_{ih}_{half}")
                        if half == 0:
                            nc.vector.tensor_copy(out=avs[0:65, :],
                                                  in_=av[half][0:65, :])
                        else:
                            nc.vector.tensor_copy(out=avs[32:64, :],
                                                  in_=av[half][32:64, :])
                            nc.vector.tensor_copy(out=avs[64:128, :],
                                                  in_=av[half][64:128, :])
                        r_l = rlp.tile([65, 1024], F32, tag="rl",
                                       name=f"rl{fc}_{ih}_{half}")
                        nc.vector.reciprocal(r_l[srow:srow + 1, :],
                                             avs[srow:srow + 1, :])
                        nc.sync.dma_start(out=d_rl[h, ih, :], in_=r_l[srow:srow + 1, :])
                        rbc = rbp.tile([128, 1024], F32, tag="rbc",
                                       name=f"rbc{fc}_{ih}_{half}")
                        nc.sync.dma_start(
                            out=rbc[po:po + 64, :],
                            in_=bass.AP(tensor=d_rl, offset=(h * IH + ih) * 1024,
                                        ap=[[0, 64], [1, 1024]]))
                        nc.vector.tensor_mul(yT[po:po + 64, fc, isl],
                                             avs[po:po + 64, :],
                                             rbc[po:po + 64, :])

            # ================= Phase 3: output projection ===================
            if PHASES < 3:
                nc.sync.dma_start(out=o_FT[:].rearrange('c n -> (c n)')[0:C * N // (2 if qkv_bf16 else 1)].bitcast(XDT), in_=i_xT[:].rearrange('m c kc n -> (m c kc n)'))
            for cc in range(KC if PHASES >= 3 else 0):
                for nt in range(4):
                    fp = psp.tile([128, 512], F32,
                                  tag=("sc0", "sc1", "avA", "avB")[(cc * 4 + nt) % 4],
                                  name=f"fp{cc}_{nt}")
                    for kc in range(4):
                        nc.tensor.matmul(
                            fp[:], wp_sb[:, kc, cc * 128:(cc + 1) * 128],
                            yT[:, kc, nt * 512:(nt + 1) * 512],
                            start=(kc == 0), stop=(kc == 3))
                    so = outp.tile([128, 512], F32, tag="so")
                    if with_proj_bias:
                        nc.scalar.activation(so[:], fp[:], AF.Identity,
                                             bias=pb_sb[:, cc:cc + 1])
                    else:
                        nc.vector.tensor_copy(out=so[:], in_=fp[:])
                    nc.sync.dma_start(
                        out=o_FT[cc * 128:(cc + 1) * 128,
                                 nt * 512:(nt + 1) * 512],
                        in_=so[:])

    nc.compile()
    return nc


def _host_prep(x, qkv_w, qkv_b, proj_w, proj_b, qn_w, kn_w, rope_cos, rope_sin,
               qkv_bf16=True):
    xdt = ml_dtypes.bfloat16 if qkv_bf16 else np.float32
    """Build the 8 per-core input maps."""
    x = np.asarray(x, dtype=np.float32)
    qkv_w = np.asarray(qkv_w, dtype=np.float32)
    qkv_b = np.asarray(qkv_b, dtype=np.float32)
    proj_w = np.asarray(proj_w, dtype=np.float32)
    proj_b = np.asarray(proj_b, dtype=np.float32)
    qn_w = np.asarray(qn_w, dtype=np.float32)
    kn_w = np.asarray(kn_w, dtype=np.float32)
    rope_cos = np.asarray(rope_cos, dtype=np.float32)
    rope_sin = np.asarray(rope_sin, dtype=np.float32)

    # rope tables with rmsnorm weight folded in:
    # out[0:32]  = xh[0:32]*(w*cos)[0:32]  + xh[32:64]*(-w2*sin[0:32])
    # out[32:64] = xh[32:64]*(w*cos)[32:64] + xh[0:32]*( w1*sin[32:64])
    def tables(w):
        a = rope_cos * w[None, :]
        b = np.empty_like(rope_sin)
        b[:, 0:32] = -rope_sin[:, 0:32] * w[None, 32:64]
        b[:, 32:64] = rope_sin[:, 32:64] * w[None, 0:32]
        return (np.ascontiguousarray(a).astype(ml_dtypes.bfloat16),
                np.ascontiguousarray(b).astype(ml_dtypes.bfloat16))

    raq, rbq = tables(qn_w)
    rak, rbk = tables(kn_w)
    with_qkv_bias = bool(np.any(qkv_b))
    with_proj_bias = bool(np.any(proj_b))
    ones1 = np.ones((1, 128), dtype=np.float32)
    pb = np.ascontiguousarray(proj_b.reshape(KC, 128).T)

    in_maps = []
    for ci in range(8):
        b, hh = divmod(ci, 2)
        fsl = slice(hh * FL, hh * FL + FL)
        m = {
            "xT": np.ascontiguousarray(
                x[b].T.reshape(KC, 128, NCHUNK, 128).transpose(2, 1, 0, 3)
            ).astype(xdt),
            "wq": np.ascontiguousarray(qkv_w[fsl, :].T).astype(xdt),
            "wk": np.ascontiguousarray(qkv_w[C:][fsl, :].T).astype(xdt),
            "wv": np.ascontiguousarray(qkv_w[2 * C:][fsl, :].T).astype(xdt),
            "wp": np.ascontiguousarray(proj_w[:, fsl].T).astype(ml_dtypes.bfloat16),
            "raq": raq, "rbq": rbq, "rak": rak, "rbk": rbk,
        }
        if with_qkv_bias:
            m["qkvb"] = np.concatenate(
                [qkv_b[fsl], qkv_b[C:][fsl], qkv_b[2 * C:][fsl]]
            ).reshape(1, 3 * FL).astype(xdt)
            m["ones1"] = ones1.astype(xdt)
        if with_proj_bias:
            m["pb"] = pb
        in_maps.append(m)
    return in_maps, with_qkv_bias, with_proj_bias


def kernel(x, qkv_w, qkv_b, proj_w, proj_b, qn_w, kn_w, rope_cos, rope_sin,
           _trace=False):
    global _PROGRAM
    in_maps, wqb, wpb = _host_prep(x, qkv_w, qkv_b, proj_w, proj_b, qn_w, kn_w,
                                   rope_cos, rope_sin)
    if _PROGRAM is None or _PROGRAM[0] != (wqb, wpb):
        _PROGRAM = ((wqb, wpb), _build_program(wqb, wpb))
    nc = _PROGRAM[1]
    kwargs = {}
    if _trace:
        kwargs = dict(trace=True, trace_cores=[0])
    res = run_bass_kernel_spmd(nc, in_maps, core_ids=list(range(8)), **kwargs)
    if _trace:
        kernel.last_exec_ns = res.exec_time_ns
        kernel.last_results = res
    out = np.empty((B, N, C), dtype=np.float32)
    for b in range(B):
        ft = res.results[2 * b]["FT"] + res.results[2 * b + 1]["FT"]
        out[b] = ft.T
    return out



# revision 6
# speedup vs baseline: 1.2346x; 1.2346x over previous
"""Fused multi-head attention block (qkv + rmsnorm + rope + sdpa + proj) for
Trainium2, sharded over 8 NeuronCores as batch x head-half (Megatron-style).

Shapes (hardcoded): B=4, N=2048, C=1024, H=16, D=64.
Each core handles one batch and 8 heads (= 512 feature columns).
Host folds qn_w/kn_w into rope tables, sums the two per-batch partial
projection outputs and transposes back to [B, N, C].
"""
import os
import sys

os.environ.setdefault("NEURON_RT_RESET_CORES", "1")
sys.path.insert(0, "/opt/trn_rl_repo")

import ml_dtypes
import numpy as np

import concourse.bass as bass
import concourse.mybir as mybir
import concourse.tile as tile
from concourse import bacc
from concourse.bass_utils import run_bass_kernel_spmd
from concourse.masks import make_identity

dt = mybir.dt
F32 = dt.float32
F32R = dt.float32r
BF16 = dt.bfloat16
AF = mybir.ActivationFunctionType

B, N, C, H, D = 4, 2048, 1024, 16, 64
HL = H // 2          # heads per core = 8
FL = HL * D          # local features = 512
EPS = 1e-6
SCALE = 1.0 / np.sqrt(D)
NCHUNK = N // 128    # 16
KC = C // 128        # 8  (c_in chunks)
IH = 2               # i-halves of 1024 in phase 2

_PROGRAM = None


def _force_combined_ln_exp_tables():
    """Make the act-table-load pass put Ln and Exp on the shared
    natural_log_exp_and_others set (instead of alternating between the
    natural_log and exp_and_others sets, ~1.3us per swap). The cached
    dict is what bacc feeds the rust pass; set ids stay aligned with
    act_info.json because we only mutate membership, not order."""
    from concourse.hw_specs import get_activation_tables
    tables = get_activation_tables("gen3")
    combined = tables.get("natural_log_exp_and_others")
    if not combined:
        return
    for name, funcs in tables.items():
        if name != "natural_log_exp_and_others":
            funcs.discard(AF.Ln)
            funcs.discard(AF.Exp)


def _build_program(with_qkv_bias, with_proj_bias, bench_reps=0, qkv_bf16=True):
    XDT = BF16 if qkv_bf16 else F32R
    xlb, nqb, scb, ptb, avb = (5, 6, 10, 4, 2) if qkv_bf16 else (3, 4, 8, 3, 1)
    nc = bacc.Bacc("TRN2", target_bir_lowering=False, debug=False, num_devices=8)
    _force_combined_ln_exp_tables()

    i_xT = nc.dram_tensor("xT", [NCHUNK, 128, KC, 128], XDT, kind="ExternalInput")
    i_wq = nc.dram_tensor("wq", [C, FL], XDT, kind="ExternalInput")
    i_wk = nc.dram_tensor("wk", [C, FL], XDT, kind="ExternalInput")
    i_wv = nc.dram_tensor("wv", [C, FL], XDT, kind="ExternalInput")
    i_wp = nc.dram_tensor("wp", [FL, C], BF16, kind="ExternalInput")
    if with_qkv_bias:
        i_qkvb = nc.dram_tensor("qkvb", [1, 3 * FL], XDT, kind="ExternalInput")
        i_ones1 = nc.dram_tensor("ones1", [1, 128], XDT, kind="ExternalInput")
    i_raq = nc.dram_tensor("raq", [N, D], BF16, kind="ExternalInput")
    i_rbq = nc.dram_tensor("rbq", [N, D], BF16, kind="ExternalInput")
    i_rak = nc.dram_tensor("rak", [N, D], BF16, kind="ExternalInput")
    i_rbk = nc.dram_tensor("rbk", [N, D], BF16, kind="ExternalInput")
    if with_proj_bias:
        i_pb = nc.dram_tensor("pb", [128, KC], F32, kind="ExternalInput")
    o_FT = nc.dram_tensor("FT", [C, N], F32, kind="ExternalOutput")

    from contextlib import ExitStack
    with tile.TileContext(nc) as tc:
        with ExitStack() as ctx:
            pp = ctx.enter_context(tc.tile_pool(name="persist", bufs=1))
            wpool = ctx.enter_context(tc.tile_pool(name="wpool", bufs=1))
            xload = ctx.enter_context(tc.tile_pool(name="xload", bufs=xlb))
            scratch = ctx.enter_context(tc.tile_pool(name="scratch", bufs=scb))
            natq = ctx.enter_context(tc.tile_pool(name="natq", bufs=nqb))
            natk = ctx.enter_context(tc.tile_pool(name="natk", bufs=nqb))
            sumsp = ctx.enter_context(tc.tile_pool(name="sumsp", bufs=4))
            ptp = ctx.enter_context(tc.tile_pool(name="ptp", bufs=ptb))
            rlp = ctx.enter_context(tc.tile_pool(name="rlp", bufs=2))
            rbp = ctx.enter_context(tc.tile_pool(name="rbp", bufs=2))
            outp = ctx.enter_context(tc.tile_pool(name="outp", bufs=2))
            # one PSUM pool, 4 tags x 2 banks (8 banks total), shared by all
            # phases: scA/scB + avA/avB
            psp = ctx.enter_context(tc.tile_pool(name="psp", bufs=1, space="PSUM"))

            # ---- persistent tensors ----
            w_sb = {}
            for nm, src in (("q", i_wq), ("k", i_wk), ("v", i_wv)):
                t = wpool.tile([128, KC, FL], XDT, tag=f"w{nm}")
                nc.sync.dma_start(out=t[:], in_=src[:].rearrange(
                    "(kc c) f -> c kc f", c=128))
                w_sb[nm] = t
            if with_qkv_bias:
                qkvb_sb = wpool.tile([1, 3 * FL], XDT, tag="qkvb")
                nc.sync.dma_start(out=qkvb_sb[:], in_=i_qkvb[:])
                ones1 = wpool.tile([1, 128], XDT, tag="ones1")
                nc.sync.dma_start(out=ones1[:], in_=i_ones1[:])
            if with_proj_bias:
                pb_sb = wpool.tile([128, KC], F32, tag="pb")
                nc.sync.dma_start(out=pb_sb[:], in_=i_pb[:])
            eps_sb = wpool.tile([128, 1], F32, tag="eps")
            nc.vector.memset(eps_sb[:], EPS)
            ident = wpool.tile([128, 128], BF16, tag="ident")
            make_identity(nc, ident[:])

            qT = pp.tile([128, 4, N], BF16, tag="qT")     # [f%128, fc, n]
            kT = pp.tile([128, 4, N], BF16, tag="kT")
            yT = pp.tile([128, 4, N], BF16, tag="yT")
            vball = pp.tile([128, NCHUNK, HL * 96], BF16, tag="vball")
            # per-head 96-col block: [1 | zeros(31) | v(64)].  Even head h
            # slices [96h+32, 96h+160): oT rows 0-63, sums row 64.  Odd head h
            # slices [96h-32, 96h+96): oT rows 64-127, sums row 32.  Unused
            # psum rows collect garbage and are never read.
            vview = vball[:].rearrange("p jc (h c) -> p jc h c", c=96)
            nc.vector.memset(vview[:, :, :, 0:32], 0.0)
            nc.vector.memset(vview[:, :, :, 0:1], 1.0)

            rope_sb = {}
            for nm2, srct in (("raq", i_raq), ("rbq", i_rbq),
                              ("rak", i_rak), ("rbk", i_rbk)):
                rt = wpool.tile([128, NCHUNK, D], BF16, tag=nm2)
                nc.sync.dma_start(out=rt[:], in_=srct[:].rearrange(
                    "(m p) d -> p m d", p=128))
                rope_sb[nm2] = rt

            wp_sb = wpool.tile([128, 4, C], BF16, tag="wp")
            nc.sync.dma_start(out=wp_sb[:], in_=i_wp[:].rearrange(
                "(kc c) o -> c kc o", c=128))

            # ================= Phase 1: qkv + rmsnorm + rope + transpose ====
            # Software-pipelined across n-chunks: S1 (qkv matmuls + psum
            # copies), S2 (rms stats + rope), S3 (PE transposes) are emitted
            # with a skew so each engine's FIFO interleaves iterations.
            loop_ctx = tc.For_i(0, bench_reps, 1) if bench_reps else None
            if loop_ctx is not None:
                ctx.enter_context(loop_ctx)

            nat_t = {}
            ro_t = {}

            def stage1(m):
                nsl = slice(m * 128, (m + 1) * 128)
                xm = xload.tile([128, KC, 128], XDT, tag="xm", name=f"xm{m}")
                nc.sync.dma_start(out=xm[:], in_=i_xT[m])
                nat = {"q": natq.tile([128, FL], BF16, tag="qnat", name=f"qnat{m}"),
                       "k": natk.tile([128, FL], BF16, tag="knat", name=f"knat{m}")}
                nat_t[m] = nat
                for ti, nm in enumerate(("q", "k", "v")):
                    for ft in range(2):
                        fsl = slice(ft * 256, (ft + 1) * 256)
                        psum = psp.tile([128, 256], F32, tag=f"sc{ft}",
                                        name=f"qkvps{m}_{ti}_{ft}")
                        if with_qkv_bias:
                            nc.tensor.matmul(
                                psum[:], ones1[:],
                                qkvb_sb[:, ti * FL + ft * 256:
                                        ti * FL + (ft + 1) * 256],
                                start=True, stop=False)
                        for kc in range(KC):
                            nc.tensor.matmul(psum[:], xm[:, kc, :],
                                             w_sb[nm][:, kc, fsl],
                                             start=(kc == 0 and not with_qkv_bias),
                                             stop=(kc == KC - 1))
                        if nm == "v":
                            # straight into AV layout [n_p, jc=m, head, d]
                            nc.scalar.copy(
                                out=vview[:, m, ft * 4:(ft + 1) * 4, 32:96],
                                in_=psum[:].rearrange("p (h d) -> p h d", h=4))
                        else:
                            nc.scalar.copy(out=nat[nm][:, fsl], in_=psum[:])

            rstd_t = {}

            def stage2a(m):
                nat = nat_t[m]
                sums = sumsp.tile([128, 2 * HL], F32, tag="sums",
                                  name=f"sums{m}")
                for si, nm in enumerate(("q", "k")):
                    sq = scratch.tile([128, FL], BF16, tag="sxt",
                                      name=f"sq_{m}_{si}")
                    nc.gpsimd.tensor_mul(sq[:], nat[nm][:], nat[nm][:])
                    nc.vector.tensor_reduce(
                        sums[:, si * HL:(si + 1) * HL],
                        sq[:].rearrange("p (h d) -> p h d", h=HL),
                        axis=mybir.AxisListType.X, op=mybir.AluOpType.add)
                rstd_t[m] = sums

            def stage2b(m):
                nsl = slice(m * 128, (m + 1) * 128)
                nat = nat_t.pop(m)
                sums = rstd_t.pop(m)
                lns = sumsp.tile([128, 2 * HL], F32, tag="lns", name=f"lns{m}")
                nc.scalar.activation(lns[:], sums[:], AF.Ln,
                                     scale=1.0 / D, bias=eps_sb[:])
                rstd = sumsp.tile([128, 2 * HL], BF16, tag="rstd",
                                  name=f"rstd{m}")
                nc.scalar.activation(rstd[:], lns[:], AF.Exp, scale=-0.5)
                for si, (nm, ra, rb) in enumerate(
                        (("q", "raq", "rbq"), ("k", "rak", "rbk"))):
                    rat = rope_sb[ra][:, m, :]
                    rbt = rope_sb[rb][:, m, :]

                    xv = nat[nm][:].rearrange("p (h d) -> p h d", h=HL)
                    rsview = bass.AP(
                        tensor=rstd.tensor, offset=rstd[:].offset + si * HL,
                        ap=[rstd[:].ap[0], [1, HL], [0, D]])
                    xn = scratch.tile([128, FL], BF16, tag="sxt",
                                      name=f"xn_{m}_{si}")
                    xnv = xn[:].rearrange("p (h d) -> p h d", h=HL)
                    nc.vector.tensor_mul(xnv, xv, rsview)

                    rav = bass.AP(tensor=rat.tensor, offset=rat.offset,
                                  ap=[rat.ap[0], [0, HL], [1, D]])
                    t1 = scratch.tile([128, FL], BF16, tag="sxt",
                                      name=f"t1_{m}_{si}")
                    t1v = t1[:].rearrange("p (h d) -> p h d", h=HL)
                    nc.vector.tensor_mul(t1v, xnv, rav)

                    ro = scratch.tile([128, FL], BF16, tag="ro", bufs=4,
                                      name=f"ro_{m}_{si}")
                    rov = ro[:].rearrange("p (h d) -> p h d", h=HL)
                    rb_lo = bass.AP(tensor=rbt.tensor, offset=rbt.offset,
                                    ap=[rbt.ap[0], [0, HL], [1, 32]])
                    rb_hi = bass.AP(tensor=rbt.tensor, offset=rbt.offset + 32,
                                    ap=[rbt.ap[0], [0, HL], [1, 32]])
                    nc.vector.tensor_mul(rov[:, :, 0:32], xnv[:, :, 32:64], rb_lo)
                    nc.vector.tensor_mul(rov[:, :, 32:64], xnv[:, :, 0:32], rb_hi)
                    nc.gpsimd.tensor_add(ro[:], ro[:], t1[:])
                    ro_t[(m, si)] = ro

            def stage3(m):
                nsl = slice(m * 128, (m + 1) * 128)
                for si, dst in ((0, qT), (1, kT)):
                    ro = ro_t.pop((m, si))
                    for fc in range(4):
                        tp = psp.tile([128, 128], BF16, tag=f"av{'AB'[fc % 2]}",
                                      name=f"trps{m}_{si}_{fc}")
                        nc.tensor.transpose(tp[:], ro[:, fc * 128:(fc + 1) * 128],
                                            ident[:])
                        nc.vector.tensor_copy(out=dst[:, fc, nsl], in_=tp[:])

            for t in range(NCHUNK + 3):
                if t >= 3:
                    stage3(t - 3)
                if 2 <= t < NCHUNK + 2:
                    stage2b(t - 2)
                if 1 <= t < NCHUNK + 1:
                    stage2a(t - 1)
                if t < NCHUNK:
                    stage1(t)

            # ================= Phase 2: attention per head-pair =============
            PHASES = int(os.environ.get("KERNEL_PHASES", "3"))
            for fc in range(4 if PHASES >= 2 else 0):
                for ih in range(IH):
                    isl = slice(ih * 1024, (ih + 1) * 1024)
                    av = {0: psp.tile([128, 1024], F32, tag="avA", name=f"avA{fc}_{ih}"),
                          1: psp.tile([128, 1024], F32, tag="avB", name=f"avB{fc}_{ih}")}
                    # per jc: scores (both halves) -> exps -> AVs of the
                    # previous jc (one-step skew keeps PE off ACT's critical
                    # path; pt bufs=3 covers the extra lifetime)
                    pt_t = {}

                    def do_av(jc):
                        for half in range(2):
                            h = 2 * fc + half
                            vs = 96 * h + 32 if half == 0 else 96 * h - 32
                            pt = pt_t.pop((jc, half))
                            for i2 in range(2):
                                nc.tensor.matmul(
                                    av[half][:, i2 * 512:(i2 + 1) * 512],
                                    vball[:, jc, vs:vs + 128],
                                    pt[:, i2 * 512:(i2 + 1) * 512],
                                    start=(jc == 0), stop=(jc == NCHUNK - 1))

                    for jc in range(NCHUNK):
                        jsl = slice(jc * 128, (jc + 1) * 128)
                        scs = []
                        for half in range(2):
                            po = 64 * half
                            sc = psp.tile([128, 1024], F32, tag=f"sc{half}",
                                          name=f"sc{fc}_{ih}_{jc}_{half}")
                            lhs = kT[po:po + 64, fc, jsl]
                            for i2 in range(2):
                                nc.tensor.matmul(
                                    sc[:, i2 * 512:(i2 + 1) * 512], lhs,
                                    qT[po:po + 64, fc,
                                       ih * 1024 + i2 * 512:
                                       ih * 1024 + (i2 + 1) * 512],
                                    start=True, stop=True)
                            scs.append(sc)
                        if os.environ.get("KERNEL_NOSM"):
                            continue
                        for half in range(2):
                            pt = ptp.tile([128, 1024], BF16, tag=f"pt{half}",
                                          name=f"pt{fc}_{ih}_{jc}_{half}")
                            nc.scalar.activation(pt[:], scs[half][:], AF.Exp,
                                                 scale=float(SCALE))
                            pt_t[(jc, half)] = pt
                        if jc > 0:
                            do_av(jc - 1)
                    if not os.environ.get("KERNEL_NOSM"):
                        do_av(NCHUNK - 1)
                    # normalize: yT[f, n] = av_oT[f, n] * (1 / av_sums[n]).
                    # even head: oT rows 0-63, sums row 64; odd head: oT rows
                    # 64-127, sums row 32 (lane-aligned by vball layout).
                    # Copy BOTH halves' AV psum -> sbuf first (frees all 4 av
                    # psum banks for the next i-half ASAP), then run the
                    # reciprocal (fast DVE approx) + gpsimd partition
                    # broadcast + final mult off the psum critical path.
                    if not os.environ.get("KERNEL_NONORM"):
                        avs_t = {}
                        for half in range(2):
                            srow = 64 if half == 0 else 32
                            avs = rlp.tile([128, 1024], F32, tag="avs", bufs=avb,
                                           name=f"avs{fc}_{ih}_{half}")
                            if half == 0:
                                nc.vector.tensor_copy(out=avs[0:65, :],
                                                      in_=av[half][0:65, :])
                            else:
                                nc.vector.tensor_copy(out=avs[32:33, :],
                                                      in_=av[half][32:33, :])
                                nc.vector.tensor_copy(out=avs[64:128, :],
                                                      in_=av[half][64:128, :])
                            avs_t[half] = avs
                        for half in range(2):
                            po = 64 * half
                            srow = 64 if half == 0 else 32
                            avs = avs_t.pop(half)
                            # reciprocal_approx_fast and partition_broadcast
                            # both require base-partition-0 inputs; stage the
                            # sums row at partition 0 first.
                            s_0 = rlp.tile([1, 1024], F32, tag="s0", bufs=2,
                                           name=f"s0_{fc}_{ih}_{half}")
                            nc.vector.tensor_copy(out=s_0[:],
                                                  in_=avs[srow:srow + 1, :])
                            r_l = rlp.tile([1, 1024], F32, tag="rl", bufs=2,
                                           name=f"rl{fc}_{ih}_{half}")
                            nc.vector.reciprocal_approx_fast(
                                out=r_l[:], in_=s_0[:])
                            # partition_broadcast misbehaves for out base
                            # partitions > 0; broadcast to all 128 rows and
                            # slice (value is row-invariant anyway).
                            rbc = rbp.tile([128, 1024], F32, tag="rbc",
                                           name=f"rbc{fc}_{ih}_{half}")
                            nc.gpsimd.partition_broadcast(
                                rbc[:, :], r_l[:], channels=128)
                            nc.vector.tensor_mul(yT[po:po + 64, fc, isl],
                                                 avs[po:po + 64, :],
                                                 rbc[po:po + 64, :])

            # ================= Phase 3: output projection ===================
            if PHASES < 3:
                nc.sync.dma_start(out=o_FT[:].rearrange('c n -> (c n)')[0:C * N // (2 if qkv_bf16 else 1)].bitcast(XDT), in_=i_xT[:].rearrange('m c kc n -> (m c kc n)'))
            for cc in range(KC if PHASES >= 3 else 0):
                for nt in range(4):
                    fp = psp.tile([128, 512], F32,
                                  tag=("sc0", "sc1", "avA", "avB")[(cc * 4 + nt) % 4],
                                  name=f"fp{cc}_{nt}")
                    for kc in range(4):
                        nc.tensor.matmul(
                            fp[:], wp_sb[:, kc, cc * 128:(cc + 1) * 128],
                            yT[:, kc, nt * 512:(nt + 1) * 512],
                            start=(kc == 0), stop=(kc == 3))
                    so = outp.tile([128, 512], F32, tag="so")
                    if with_proj_bias:
                        nc.scalar.activation(so[:], fp[:], AF.Identity,
                                             bias=pb_sb[:, cc:cc + 1])
                    else:
                        nc.vector.tensor_copy(out=so[:], in_=fp[:])
                    nc.sync.dma_start(
                        out=o_FT[cc * 128:(cc + 1) * 128,
                                 nt * 512:(nt + 1) * 512],
                        in_=so[:])

    nc.compile()
    return nc


def _host_prep(x, qkv_w, qkv_b, proj_w, proj_b, qn_w, kn_w, rope_cos, rope_sin,
               qkv_bf16=True):
    xdt = ml_dtypes.bfloat16 if qkv_bf16 else np.float32
    """Build the 8 per-core input maps."""
    x = np.asarray(x, dtype=np.float32)
    qkv_w = np.asarray(qkv_w, dtype=np.float32)
    qkv_b = np.asarray(qkv_b, dtype=np.float32)
    proj_w = np.asarray(proj_w, dtype=np.float32)
    proj_b = np.asarray(proj_b, dtype=np.float32)
    qn_w = np.asarray(qn_w, dtype=np.float32)
    kn_w = np.asarray(kn_w, dtype=np.float32)
    rope_cos = np.asarray(rope_cos, dtype=np.float32)
    rope_sin = np.asarray(rope_sin, dtype=np.float32)

    # rope tables with rmsnorm weight folded in:
    # out[0:32]  = xh[0:32]*(w*cos)[0:32]  + xh[32:64]*(-w2*sin[0:32])
    # out[32:64] = xh[32:64]*(w*cos)[32:64] + xh[0:32]*( w1*sin[32:64])
    def tables(w):
        a = rope_cos * w[None, :]
        b = np.empty_like(rope_sin)
        b[:, 0:32] = -rope_sin[:, 0:32] * w[None, 32:64]
        b[:, 32:64] = rope_sin[:, 32:64] * w[None, 0:32]
        return (np.ascontiguousarray(a).astype(ml_dtypes.bfloat16),
                np.ascontiguousarray(b).astype(ml_dtypes.bfloat16))

    raq, rbq = tables(qn_w)
    rak, rbk = tables(kn_w)
    with_qkv_bias = bool(np.any(qkv_b))
    with_proj_bias = bool(np.any(proj_b))
    ones1 = np.ones((1, 128), dtype=np.float32)
    pb = np.ascontiguousarray(proj_b.reshape(KC, 128).T)

    in_maps = []
    for ci in range(8):
        b, hh = divmod(ci, 2)
        fsl = slice(hh * FL, hh * FL + FL)
        m = {
            "xT": np.ascontiguousarray(
                x[b].T.reshape(KC, 128, NCHUNK, 128).transpose(2, 1, 0, 3)
            ).astype(xdt),
            "wq": np.ascontiguousarray(qkv_w[fsl, :].T).astype(xdt),
            "wk": np.ascontiguousarray(qkv_w[C:][fsl, :].T).astype(xdt),
            "wv": np.ascontiguousarray(qkv_w[2 * C:][fsl, :].T).astype(xdt),
            "wp": np.ascontiguousarray(proj_w[:, fsl].T).astype(ml_dtypes.bfloat16),
            "raq": raq, "rbq": rbq, "rak": rak, "rbk": rbk,
        }
        if with_qkv_bias:
            m["qkvb"] = np.concatenate(
                [qkv_b[fsl], qkv_b[C:][fsl], qkv_b[2 * C:][fsl]]
            ).reshape(1, 3 * FL).astype(xdt)
            m["ones1"] = ones1.astype(xdt)
        if with_proj_bias:
            m["pb"] = pb
        in_maps.append(m)
    return in_maps, with_qkv_bias, with_proj_bias


def kernel(x, qkv_w, qkv_b, proj_w, proj_b, qn_w, kn_w, rope_cos, rope_sin,
           _trace=False):
    global _PROGRAM
    in_maps, wqb, wpb = _host_prep(x, qkv_w, qkv_b, proj_w, proj_b, qn_w, kn_w,
                                   rope_cos, rope_sin)
    if _PROGRAM is None or _PROGRAM[0] != (wqb, wpb):
        _PROGRAM = ((wqb, wpb), _build_program(wqb, wpb))
    nc = _PROGRAM[1]
    kwargs = {}
    if _trace:
        kwargs = dict(trace=True, trace_cores=[0])
    res = run_bass_kernel_spmd(nc, in_maps, core_ids=list(range(8)), **kwargs)
    if _trace:
        kernel.last_exec_ns = res.exec_time_ns
        kernel.last_results = res
    out = np.empty((B, N, C), dtype=np.float32)
    for b in range(B):
        ft = res.results[2 * b]["FT"] + res.results[2 * b + 1]["FT"]
        out[b] = ft.T
    return out



# revision 17
# speedup vs baseline: 1.2427x; 1.0066x over previous
"""Fused multi-head attention block (qkv + rmsnorm + rope + sdpa + proj) for
Trainium2, sharded over 8 NeuronCores as batch x head-half (Megatron-style).

Shapes (hardcoded): B=4, N=2048, C=1024, H=16, D=64.
Each core handles one batch and 8 heads (= 512 feature columns).

Phase 1 computes q/k directly in transposed [feature, n] layout (weights
stationary), so no PE transposes are needed; the rms-norm d-reduction becomes
a mask matmul over partitions, rsqrt is Ln/Exp on ACT, and the per-(head, n)
rstd is broadcast down the 64 partitions of each head via a DRAM bounce.
RoPE pairs (d, d+32) are stored adjacently (permuted row order, which is
sound because scores contract over d), so the rotate-half becomes a
stream_shuffle pair swap. V is computed in natural [n, feature] layout
straight into the AV operand tile.
"""
import os
import sys

os.environ.setdefault("NEURON_RT_RESET_CORES", "1")
sys.path.insert(0, "/opt/trn_rl_repo")

import ml_dtypes
import numpy as np

import concourse.bass as bass
import concourse.mybir as mybir
import concourse.tile as tile
from concourse import bacc
from concourse.bass_utils import run_bass_kernel_spmd

dt = mybir.dt
F32 = dt.float32
BF16 = dt.bfloat16
AF = mybir.ActivationFunctionType

B, N, C, H, D = 4, 2048, 1024, 16, 64
HL = H // 2          # heads per core = 8
FL = HL * D          # local features = 512
EPS = 1e-6
SCALE = 1.0 / np.sqrt(D)
NCHUNK = N // 128    # 16
KC = C // 128        # 8  (c_in chunks)
NT = 4               # n-blocks of 512 in phase 1
IH = 2               # i-halves of 1024 in phase 2

# within each head's 64 rows, row p holds dim d(p) so that rope partners
# (d, d+32) sit on adjacent partitions (swappable by stream_shuffle)
DMAP = np.array([(p % 2) * 32 + p // 2 for p in range(64)])
# feature permutation for the local q/k weight columns (per-head DMAP)
FPERM = np.concatenate([hl * 64 + DMAP for hl in range(HL)])
PAIRSWAP = [(2 * i + 1, 2 * i) for i in range(16)]
PAIRSWAP = [x for pr in PAIRSWAP for x in pr]

_PROGRAM = None


def _force_combined_ln_exp_tables():
    """Make the act-table-load pass put Ln and Exp on the shared
    natural_log_exp_and_others set (instead of alternating between the
    natural_log and exp_and_others sets, ~1.3us per swap). The cached
    dict is what bacc feeds the rust pass; set ids stay aligned with
    act_info.json because we only mutate membership, not order."""
    from concourse.hw_specs import get_activation_tables
    tables = get_activation_tables("gen3")
    combined = tables.get("natural_log_exp_and_others")
    if not combined:
        return
    for name, funcs in tables.items():
        if name != "natural_log_exp_and_others":
            funcs.discard(AF.Ln)
            funcs.discard(AF.Exp)


def _build_program(with_qkv_bias, with_proj_bias, shared_tables):
    nc = bacc.Bacc("TRN2", target_bir_lowering=False, debug=False, num_devices=8)
    _force_combined_ln_exp_tables()

    i_xT = nc.dram_tensor("xT", [128, NT, KC, 512], BF16, kind="ExternalInput")
    i_wq = nc.dram_tensor("wq", [C, FL], BF16, kind="ExternalInput")
    i_wk = nc.dram_tensor("wk", [C, FL], BF16, kind="ExternalInput")
    i_wv = nc.dram_tensor("wv", [C, FL], BF16, kind="ExternalInput")
    i_wp = nc.dram_tensor("wp", [FL, C], BF16, kind="ExternalInput")
    i_raq = nc.dram_tensor("raq", [128, N], BF16, kind="ExternalInput")
    i_rbq = nc.dram_tensor("rbq", [128, N], BF16, kind="ExternalInput")
    if not shared_tables:
        i_rak = nc.dram_tensor("rak", [128, N], BF16, kind="ExternalInput")
        i_rbk = nc.dram_tensor("rbk", [128, N], BF16, kind="ExternalInput")
    if with_qkv_bias:
        i_qkbT = nc.dram_tensor("qkbT", [128, 8], F32, kind="ExternalInput")
        i_vb = nc.dram_tensor("vb", [1, FL], BF16, kind="ExternalInput")
        i_ones1 = nc.dram_tensor("ones1", [1, 128], BF16, kind="ExternalInput")
    if with_proj_bias:
        i_pb = nc.dram_tensor("pb", [128, KC], F32, kind="ExternalInput")
    o_FT = nc.dram_tensor("FT", [C, N], F32, kind="ExternalOutput")
    d_rstd = nc.dram_tensor("d_rstd", [2, 4, 2, N], BF16)  # (t, fc, row, n)

    from contextlib import ExitStack
    with tile.TileContext(nc) as tc:
        with ExitStack() as ctx:
            pp = ctx.enter_context(tc.tile_pool(name="persist", bufs=1))
            wpool = ctx.enter_context(tc.tile_pool(name="wpool", bufs=1))
            scr = ctx.enter_context(tc.tile_pool(name="scr", bufs=2))
            ptp = ctx.enter_context(tc.tile_pool(name="ptp", bufs=3))
            rlp = ctx.enter_context(tc.tile_pool(name="rlp", bufs=2))
            rbp = ctx.enter_context(tc.tile_pool(name="rbp", bufs=1))
            outp = ctx.enter_context(tc.tile_pool(name="outp", bufs=2))
            psp = ctx.enter_context(tc.tile_pool(name="psp", bufs=1, space="PSUM"))

            # ---- persistent tensors ----
            w_sb = {}
            for nm, src in (("q", i_wq), ("k", i_wk), ("v", i_wv)):
                t = wpool.tile([128, KC, FL], BF16, tag=f"w{nm}")
                nc.sync.dma_start(out=t[:], in_=src[:].rearrange(
                    "(kc c) f -> c kc f", c=128))
                w_sb[nm] = t
            wp_sb = wpool.tile([128, 4, C], BF16, tag="wp")
            nc.sync.dma_start(out=wp_sb[:], in_=i_wp[:].rearrange(
                "(kc c) o -> c kc o", c=128))
            raT = wpool.tile([128, N], BF16, tag="raT")
            rbT = wpool.tile([128, N], BF16, tag="rbT")
            nc.sync.dma_start(out=raT[:], in_=i_raq[:])
            nc.sync.dma_start(out=rbT[:], in_=i_rbq[:])
            mask01 = wpool.tile([128, 2], BF16, tag="mask01")
            nc.vector.memset(mask01[:], 0.0)
            nc.vector.memset(mask01[0:64, 0:1], 1.0)
            nc.vector.memset(mask01[64:128, 1:2], 1.0)
            eps_sb = wpool.tile([128, 1], F32, tag="eps")
            nc.vector.memset(eps_sb[:], EPS)
            if with_qkv_bias:
                qkbT = wpool.tile([128, 8], F32, tag="qkbT")
                nc.sync.dma_start(out=qkbT[:], in_=i_qkbT[:])
                vb_sb = wpool.tile([1, FL], BF16, tag="vb")
                nc.sync.dma_start(out=vb_sb[:], in_=i_vb[:])
                ones1 = wpool.tile([1, 128], BF16, tag="ones1")
                nc.sync.dma_start(out=ones1[:], in_=i_ones1[:])
            if with_proj_bias:
                pb_sb = wpool.tile([128, KC], F32, tag="pb")
                nc.sync.dma_start(out=pb_sb[:], in_=i_pb[:])

            qT = pp.tile([128, 4, N], BF16, tag="qT")     # [f%128, fc, n]
            kT = pp.tile([128, 4, N], BF16, tag="kT")
            yT = pp.tile([128, 4, N], BF16, tag="yT")
            vball = pp.tile([128, NCHUNK, HL * 96], BF16, tag="vball")
            # per-head 96-col block: [1 | zeros(31) | v(64)].  Even head h
            # slices [96h+32, 96h+160): oT rows 0-63, sums row 64.  Odd head h
            # slices [96h-32, 96h+96): oT rows 64-127, sums row 32.  Unused
            # psum rows collect garbage and are never read.
            vview = vball[:].rearrange("p jc (h c) -> p jc h c", c=96)
            nc.vector.memset(vview[:, :, :, 0:32], 0.0)
            nc.vector.memset(vview[:, :, :, 0:1], 1.0)

            xT_sb = wpool.tile([128, NT, KC, 512], BF16, tag="xT")
            for nt in range(NT):
                nc.sync.dma_start(out=xT_sb[:, nt], in_=i_xT[:, nt])

            # ======== Phase 1a: q/k in transposed layout + rms + rope ======
            pend_stat = []

            def flush_stats():
                while pend_stat:
                    pend_stat.pop(0)()

            for ti, (nm, dstT) in enumerate((("q", qT), ("k", kT))):
                if ti == 1 and not shared_tables:
                    nc.sync.dma_start(out=raT[:], in_=i_rak[:])
                    nc.sync.dma_start(out=rbT[:], in_=i_rbk[:])
                for fc in range(4):
                    fsl = slice(fc * 128, (fc + 1) * 128)
                    raw = scr.tile([128, N], BF16, tag="raw",
                                   name=f"raw{ti}_{fc}")
                    st = {}
                    for h2 in range(2):
                        st[h2] = psp.tile([2, 1024], F32,
                                          tag=("avA", "avB")[h2],
                                          name=f"st{ti}_{fc}_{h2}")
                    for nt in range(NT):
                        nsl = slice(nt * 512, (nt + 1) * 512)
                        ps = psp.tile([128, 512], F32, tag=("sc0", "sc1")[nt % 2],
                                      name=f"qk{ti}_{fc}_{nt}")
                        for kc in range(KC):
                            nc.tensor.matmul(ps[:], w_sb[nm][:, kc, fsl],
                                             xT_sb[:, nt, kc, :],
                                             start=(kc == 0), stop=(kc == KC - 1))
                        if with_qkv_bias:
                            nc.scalar.activation(
                                raw[:, nsl], ps[:], AF.Identity,
                                bias=qkbT[:, ti * 4 + fc:ti * 4 + fc + 1])
                        else:
                            nc.scalar.copy(out=raw[:, nsl], in_=ps[:])
                        sq = scr.tile([128, 512], BF16, tag="t1",
                                      name=f"sq{ti}_{fc}_{nt}")
                        nc.vector.tensor_mul(sq[:], raw[:, nsl], raw[:, nsl])
                        flush_stats()
                        pend_stat.append(
                            (lambda st_=st[nt // 2], sq_=sq, c_=(nt % 2) * 512:
                             nc.tensor.matmul(st_[:, c_:c_ + 512], mask01[:],
                                              sq_[:], start=True, stop=True)))
                    flush_stats()
                    # rstd = exp(-0.5 * ln(mean(sq) + eps)); Ln reads the
                    # stats psum directly
                    lns = scr.tile([2, N], BF16, tag="lns", bufs=1,
                                   name=f"lns{ti}_{fc}")
                    for h2 in range(2):
                        nc.scalar.activation(lns[:, h2 * 1024:(h2 + 1) * 1024],
                                             st[h2][:], AF.Ln,
                                             scale=1.0 / D, bias=eps_sb[0:2])
                    rstd = scr.tile([2, N], BF16, tag="rstd", bufs=1,
                                    name=f"rstd{ti}_{fc}")
                    nc.scalar.activation(rstd[:], lns[:], AF.Exp, scale=-0.5)
                    nc.sync.dma_start(out=d_rstd[ti, fc], in_=rstd[:])
                    # rope + rstd application per n-half (1024-wide scratch)
                    for nh in range(2):
                        nsl = slice(nh * 1024, (nh + 1) * 1024)
                        rbcast = scr.tile([128, 1024], BF16, tag="rbcast",
                                          name=f"rbc{ti}_{fc}_{nh}")
                        nc.sync.dma_start(
                            out=rbcast[:],
                            in_=bass.AP(tensor=d_rstd,
                                        offset=(ti * 4 + fc) * 2 * N + nh * 1024,
                                        ap=[[N, 2], [0, 64], [1, 1024]]))
                        nc.vector.tensor_mul(raw[:, nsl], raw[:, nsl],
                                             rbcast[:])  # xn in place
                        t1 = scr.tile([128, 1024], BF16, tag="t1",
                                      name=f"t1{ti}_{fc}_{nh}")
                        nc.vector.tensor_mul(t1[:], raw[:, nsl], raT[:, nsl])
                        u = scr.tile([128, 1024], BF16, tag="ush",
                                     name=f"u{ti}_{fc}_{nh}")
                        nc.vector.tensor_mul(u[:], raw[:, nsl], rbT[:, nsl])
                        sh = scr.tile([128, 1024], BF16, tag="ush",
                                      name=f"sh{ti}_{fc}_{nh}")
                        nc.vector.stream_shuffle(sh[:], u[:], PAIRSWAP)
                        nc.gpsimd.tensor_add(dstT[:, fc, nsl], t1[:], sh[:])

            # ======== Phase 1b: v in natural layout into vball ============
            for m in range(NCHUNK):
                ps = psp.tile([128, 512], F32, tag=("sc0", "sc1")[m % 2],
                              name=f"v{m}")
                if with_qkv_bias:
                    nc.tensor.matmul(ps[:], ones1[:], vb_sb[:],
                                     start=True, stop=False)
                for kc in range(KC):
                    nc.tensor.matmul(ps[:], xT_sb[:, m // 4, kc,
                                                  (m % 4) * 128:(m % 4 + 1) * 128],
                                     w_sb["v"][:, kc, :],
                                     start=(kc == 0 and not with_qkv_bias),
                                     stop=(kc == KC - 1))
                nc.scalar.copy(out=vview[:, m, :, 32:96],
                               in_=ps[:].rearrange("p (h d) -> p h d", h=HL))

            # ================= Phase 2: attention per head-pair =============
            PHASES = int(os.environ.get("KERNEL_PHASES", "3"))
            for fc in range(4 if PHASES >= 2 else 0):
                for ih in range(IH):
                    isl = slice(ih * 1024, (ih + 1) * 1024)
                    av = {0: psp.tile([128, 1024], F32, tag="avA", name=f"avA{fc}_{ih}"),
                          1: psp.tile([128, 1024], F32, tag="avB", name=f"avB{fc}_{ih}")}
                    # per jc: scores (both halves) -> exps -> AVs of the
                    # previous jc (one-step skew keeps PE off ACT's critical
                    # path; pt bufs cover the extra lifetime)
                    pt_t = {}

                    def do_av(jc):
                        for half in range(2):
                            h = 2 * fc + half
                            vs = 96 * h + 32 if half == 0 else 96 * h - 32
                            pt = pt_t.pop((jc, half))
                            for i2 in range(2):
                                nc.tensor.matmul(
                                    av[half][:, i2 * 512:(i2 + 1) * 512],
                                    vball[:, jc, vs:vs + 128],
                                    pt[:, i2 * 512:(i2 + 1) * 512],
                                    start=(jc == 0), stop=(jc == NCHUNK - 1))

                    for jc in range(NCHUNK):
                        jsl = slice(jc * 128, (jc + 1) * 128)
                        scs = []
                        for half in range(2):
                            po = 64 * half
                            sc = psp.tile([128, 1024], F32, tag=f"sc{half}",
                                          name=f"sc{fc}_{ih}_{jc}_{half}")
                            lhs = kT[po:po + 64, fc, jsl]
                            for i2 in range(2):
                                nc.tensor.matmul(
                                    sc[:, i2 * 512:(i2 + 1) * 512], lhs,
                                    qT[po:po + 64, fc,
                                       ih * 1024 + i2 * 512:
                                       ih * 1024 + (i2 + 1) * 512],
                                    start=True, stop=True)
                            scs.append(sc)
                        if os.environ.get("KERNEL_NOSM"):
                            continue
                        for half in range(2):
                            pt = ptp.tile([128, 1024], BF16, tag=f"pt{half}",
                                          name=f"pt{fc}_{ih}_{jc}_{half}")
                            nc.scalar.activation(pt[:], scs[half][:], AF.Exp,
                                                 scale=float(SCALE))
                            pt_t[(jc, half)] = pt
                        if jc > 0:
                            do_av(jc - 1)
                    if not os.environ.get("KERNEL_NOSM"):
                        do_av(NCHUNK - 1)
                    # normalize: yT[f, n] = av_oT[f, n] * (1 / av_sums[n]).
                    # even head: oT rows 0-63, sums row 64; odd head: oT rows
                    # 64-127, sums row 32 (lane-aligned by vball layout).
                    # Copy BOTH halves' AV psum -> sbuf first (frees all 4 av
                    # psum banks for the next i-half ASAP), then run the
                    # reciprocal (fast DVE approx) + gpsimd partition
                    # broadcast + final mult off the psum critical path.
                    if not os.environ.get("KERNEL_NONORM"):
                        avs_t = {}
                        for half in range(2):
                            avs = rlp.tile([128, 1024], F32, tag="avs",
                                           name=f"avs{fc}_{ih}_{half}")
                            if half == 0:
                                nc.vector.tensor_copy(out=avs[0:65, :],
                                                      in_=av[half][0:65, :])
                            else:
                                nc.vector.tensor_copy(out=avs[32:33, :],
                                                      in_=av[half][32:33, :])
                                nc.vector.tensor_copy(out=avs[64:128, :],
                                                      in_=av[half][64:128, :])
                            avs_t[half] = avs
                        for half in range(2):
                            po = 64 * half
                            srow = 64 if half == 0 else 32
                            avs = avs_t.pop(half)
                            # reciprocal_approx_fast and partition_broadcast
                            # both require base-partition-0 inputs; stage the
                            # sums row at partition 0 first.
                            s_0 = rlp.tile([1, 1024], F32, tag="s0", bufs=1,
                                           name=f"s0_{fc}_{ih}_{half}")
                            nc.vector.tensor_copy(out=s_0[:],
                                                  in_=avs[srow:srow + 1, :])
                            r_l = rlp.tile([1, 1024], F32, tag="rl", bufs=1,
                                           name=f"rl{fc}_{ih}_{half}")
                            nc.vector.reciprocal_approx_fast(
                                out=r_l[:], in_=s_0[:])
                            r_h = rlp.tile([1, 1024], BF16, tag="rlh", bufs=1,
                                           name=f"rlh{fc}_{ih}_{half}")
                            nc.vector.tensor_copy(out=r_h[:], in_=r_l[:])
                            rbc = rbp.tile([128, 1024], BF16, tag="rbc",
                                           name=f"rbc{fc}_{ih}_{half}")
                            nc.gpsimd.partition_broadcast(
                                rbc[:, :], r_h[:], channels=128)
                            nc.vector.tensor_mul(yT[po:po + 64, fc, isl],
                                                 avs[po:po + 64, :],
                                                 rbc[po:po + 64, :])

            # ================= Phase 3: output projection ===================
            for cc in range(KC if PHASES >= 3 else 0):
                for nt in range(4):
                    fp = psp.tile([128, 512], F32,
                                  tag=("sc0", "sc1", "avA", "avB")[(cc * 4 + nt) % 4],
                                  name=f"fp{cc}_{nt}")
                    for kc in range(4):
                        nc.tensor.matmul(
                            fp[:], wp_sb[:, kc, cc * 128:(cc + 1) * 128],
                            yT[:, kc, nt * 512:(nt + 1) * 512],
                            start=(kc == 0), stop=(kc == 3))
                    so = outp.tile([128, 512], BF16, tag="so")
                    if with_proj_bias:
                        nc.scalar.activation(so[:], fp[:], AF.Identity,
                                             bias=pb_sb[:, cc:cc + 1])
                    elif (cc * 4 + nt) % 2 == 0:
                        nc.scalar.copy(out=so[:], in_=fp[:])
                    else:
                        nc.vector.tensor_copy(out=so[:], in_=fp[:])
                    # SWDGE casting DMA (bf16 -> f32) keeps the staging tile
                    # small; gpsimd is idle here
                    nc.gpsimd.dma_start(
                        out=o_FT[cc * 128:(cc + 1) * 128,
                                 nt * 512:(nt + 1) * 512],
                        in_=so[:])

    nc.compile()
    return nc


def _host_prep(x, qkv_w, qkv_b, proj_w, proj_b, qn_w, kn_w, rope_cos, rope_sin):
    """Build the 8 per-core input maps."""
    x = np.asarray(x, dtype=np.float32)
    qkv_w = np.asarray(qkv_w, dtype=np.float32)
    qkv_b = np.asarray(qkv_b, dtype=np.float32)
    proj_w = np.asarray(proj_w, dtype=np.float32)
    proj_b = np.asarray(proj_b, dtype=np.float32)
    qn_w = np.asarray(qn_w, dtype=np.float32)
    kn_w = np.asarray(kn_w, dtype=np.float32)
    rope_cos = np.asarray(rope_cos, dtype=np.float32)
    rope_sin = np.asarray(rope_sin, dtype=np.float32)

    # f-major rope tables, rows permuted by DMAP (pair-adjacent), with the
    # rms-norm weight folded in:
    #   out[p] = xn[p]*a[d(p)] + xn[p^1]*b[d(p)]
    # so the u-table row p must carry b[d(p^1)] (it lands at p^1 post-swap).
    def tables(w):
        a = rope_cos * w[None, :]                    # [N, D]
        b = np.empty_like(rope_sin)
        b[:, 0:32] = -rope_sin[:, 0:32] * w[None, 32:64]
        b[:, 32:64] = rope_sin[:, 32:64] * w[None, 0:32]
        aT = a[:, DMAP].T                            # [64, N]
        pair = DMAP[np.arange(64) ^ 1]
        bT = b[:, pair].T                            # [64, N]
        aT = np.concatenate([aT, aT], axis=0)        # [128, N]
        bT = np.concatenate([bT, bT], axis=0)
        return (np.ascontiguousarray(aT).astype(ml_dtypes.bfloat16),
                np.ascontiguousarray(bT).astype(ml_dtypes.bfloat16))

    raq, rbq = tables(qn_w)
    shared_tables = bool(np.array_equal(qn_w, kn_w))
    if not shared_tables:
        rak, rbk = tables(kn_w)
    with_qkv_bias = bool(np.any(qkv_b))
    with_proj_bias = bool(np.any(proj_b))
    pb = np.ascontiguousarray(proj_b.reshape(KC, 128).T)

    in_maps = []
    for ci in range(8):
        b, hh = divmod(ci, 2)
        fsl = slice(hh * FL, hh * FL + FL)
        xb = np.ascontiguousarray(
            x[b].T.reshape(KC, 128, NT, 512).transpose(1, 2, 0, 3)
        ).astype(ml_dtypes.bfloat16)                 # [128, NT, KC, 512]
        m = {
            "xT": xb,
            # q/k weight columns permuted so GEMM output row p holds dim
            # d(p) (matches the permuted rope tables); scores are invariant
            # because q and k use the same permutation
            "wq": np.ascontiguousarray(qkv_w[fsl, :].T[:, FPERM]).astype(ml_dtypes.bfloat16),
            "wk": np.ascontiguousarray(qkv_w[C:][fsl, :].T[:, FPERM]).astype(ml_dtypes.bfloat16),
            "wv": np.ascontiguousarray(qkv_w[2 * C:][fsl, :].T).astype(ml_dtypes.bfloat16),
            "wp": np.ascontiguousarray(proj_w[:, fsl].T).astype(ml_dtypes.bfloat16),
            "raq": raq, "rbq": rbq,
        }
        if not shared_tables:
            m["rak"] = rak
            m["rbk"] = rbk
        if with_qkv_bias:
            # per-partition bias for q/k (transposed layout, permuted rows)
            qkbT = np.zeros((128, 8), dtype=np.float32)
            for ti, off in ((0, 0), (1, C)):
                bb = qkv_b[off:][fsl]                # [FL]
                for fc in range(4):
                    blk = bb[fc * 128:(fc + 1) * 128].reshape(2, 64)
                    qkbT[0:64, ti * 4 + fc] = blk[0][DMAP]
                    qkbT[64:128, ti * 4 + fc] = blk[1][DMAP]
            m["qkbT"] = qkbT
            m["vb"] = qkv_b[2 * C:][fsl].reshape(1, FL).astype(ml_dtypes.bfloat16)
            m["ones1"] = np.ones((1, 128), dtype=ml_dtypes.bfloat16)
        if with_proj_bias:
            m["pb"] = pb
        in_maps.append(m)
    return in_maps, (with_qkv_bias, with_proj_bias, shared_tables)


def kernel(x, qkv_w, qkv_b, proj_w, proj_b, qn_w, kn_w, rope_cos, rope_sin,
           _trace=False):
    global _PROGRAM
    in_maps, key = _host_prep(x, qkv_w, qkv_b, proj_w, proj_b, qn_w, kn_w,
                              rope_cos, rope_sin)
    if _PROGRAM is None or _PROGRAM[0] != key:
        _PROGRAM = (key, _build_program(*key))
    nc = _PROGRAM[1]
    kwargs = {}
    if _trace:
        kwargs = dict(trace=True, trace_cores=[0])
    res = run_bass_kernel_spmd(nc, in_maps, core_ids=list(range(8)), **kwargs)
    if _trace:
        kernel.last_exec_ns = res.exec_time_ns
        kernel.last_results = res
    out = np.empty((B, N, C), dtype=np.float32)
    for b in range(B):
        ft = res.results[2 * b]["FT"] + res.results[2 * b + 1]["FT"]
        out[b] = ft.T
    return out


# revision 25
# speedup vs baseline: 1.4802x; 1.1911x over previous
"""Fused multi-head attention block (qkv + rmsnorm + rope + sdpa + proj) for
Trainium2, sharded over 8 NeuronCores as batch x head-half (Megatron-style).

Shapes (hardcoded): B=4, N=2048, C=1024, H=16, D=64.
Each core handles one batch and 8 heads (= 512 feature columns).

Phase 1 computes q/k directly in transposed [feature, n] layout (weights
stationary), so no PE transposes are needed; the rms-norm d-reduction becomes
a mask matmul over partitions, rsqrt is Ln/Exp on ACT, and the per-(head, n)
rstd is broadcast down the 64 partitions of each head via a DRAM bounce.
RoPE pairs (d, d+32) are stored adjacently (permuted row order, which is
sound because scores contract over d), so the rotate-half becomes a
stream_shuffle pair swap. V is computed in natural [n, feature] layout
straight into the AV operand tile.
"""
import os
import sys

os.environ.setdefault("NEURON_RT_RESET_CORES", "1")
sys.path.insert(0, "/opt/trn_rl_repo")

import ml_dtypes
import numpy as np

import concourse.bass as bass
import concourse.mybir as mybir
import concourse.tile as tile
from concourse import bacc
from concourse.bass_utils import run_bass_kernel_spmd

dt = mybir.dt
F32 = dt.float32
BF16 = dt.bfloat16
AF = mybir.ActivationFunctionType

B, N, C, H, D = 4, 2048, 1024, 16, 64
HL = H // 2          # heads per core = 8
FL = HL * D          # local features = 512
EPS = 1e-6
SCALE = 1.0 / np.sqrt(D)
NCHUNK = N // 128    # 16
KC = C // 128        # 8  (c_in chunks)
NT = 4               # n-blocks of 512 in phase 1
IH = 2               # i-halves of 1024 in phase 2

# within each head's 64 rows, row p holds dim d(p) so that rope partners
# (d, d+32) sit on adjacent partitions (swappable by stream_shuffle)
DMAP = np.array([(p % 2) * 32 + p // 2 for p in range(64)])
# feature permutation for the local q/k weight columns (per-head DMAP)
FPERM = np.concatenate([hl * 64 + DMAP for hl in range(HL)])
PAIRSWAP = [(2 * i + 1, 2 * i) for i in range(16)]
PAIRSWAP = [x for pr in PAIRSWAP for x in pr]

_PROGRAM = None


def _force_combined_ln_exp_tables():
    """Make the act-table-load pass put Ln and Exp on the shared
    natural_log_exp_and_others set (instead of alternating between the
    natural_log and exp_and_others sets, ~1.3us per swap). The cached
    dict is what bacc feeds the rust pass; set ids stay aligned with
    act_info.json because we only mutate membership, not order."""
    from concourse.hw_specs import get_activation_tables
    tables = get_activation_tables("gen3")
    combined = tables.get("natural_log_exp_and_others")
    if not combined:
        return
    for name, funcs in tables.items():
        if name != "natural_log_exp_and_others":
            funcs.discard(AF.Ln)
            funcs.discard(AF.Exp)


def _build_program(with_qkv_bias, with_proj_bias, shared_tables):
    nc = bacc.Bacc("TRN2", target_bir_lowering=False, debug=False, num_devices=8)
    _force_combined_ln_exp_tables()

    i_xT = nc.dram_tensor("xT", [128, NT, KC, 512], BF16, kind="ExternalInput")
    i_wq = nc.dram_tensor("wq", [128, KC, FL], BF16, kind="ExternalInput")
    i_wk = nc.dram_tensor("wk", [128, KC, FL], BF16, kind="ExternalInput")
    i_wv = nc.dram_tensor("wv", [128, KC, FL], BF16, kind="ExternalInput")
    i_wp = nc.dram_tensor("wp", [128, 4, C], BF16, kind="ExternalInput")
    i_raq = nc.dram_tensor("raq", [128, N], BF16, kind="ExternalInput")
    i_rbq = nc.dram_tensor("rbq", [128, N], BF16, kind="ExternalInput")
    if not shared_tables:
        i_rak = nc.dram_tensor("rak", [128, N], BF16, kind="ExternalInput")
        i_rbk = nc.dram_tensor("rbk", [128, N], BF16, kind="ExternalInput")
    if with_qkv_bias:
        i_qkbT = nc.dram_tensor("qkbT", [128, 8], F32, kind="ExternalInput")
        i_vb = nc.dram_tensor("vb", [1, FL], BF16, kind="ExternalInput")
        i_ones1 = nc.dram_tensor("ones1", [1, 128], BF16, kind="ExternalInput")
    if with_proj_bias:
        i_pb = nc.dram_tensor("pb", [128, KC], F32, kind="ExternalInput")
    o_FT = nc.dram_tensor("FT", [C, N], F32, kind="ExternalOutput")
    d_rstd = nc.dram_tensor("d_rstd", [2, 4, 2, N], BF16)  # (t, fc, row, n)

    from contextlib import ExitStack
    with tile.TileContext(nc) as tc:
        with ExitStack() as ctx:
            pp = ctx.enter_context(tc.tile_pool(name="persist", bufs=1))
            wpool = ctx.enter_context(tc.tile_pool(name="wpool", bufs=1))
            scr = ctx.enter_context(tc.tile_pool(name="scr", bufs=2))
            ptp = ctx.enter_context(tc.tile_pool(name="ptp", bufs=3))
            rlp = ctx.enter_context(tc.tile_pool(name="rlp", bufs=2))
            rbp = ctx.enter_context(tc.tile_pool(name="rbp", bufs=1))
            outp = ctx.enter_context(tc.tile_pool(name="outp", bufs=2))
            psp = ctx.enter_context(tc.tile_pool(name="psp", bufs=1, space="PSUM"))

            # ---- persistent tensors ----
            w_sb = {}
            for nm, src in (("q", i_wq), ("k", i_wk), ("v", i_wv)):
                t = wpool.tile([128, KC, FL], BF16, tag=f"w{nm}")
                nc.sync.dma_start(out=t[:], in_=src[:])
                w_sb[nm] = t
            wp_sb = wpool.tile([128, 4, C], BF16, tag="wp")
            nc.sync.dma_start(out=wp_sb[:], in_=i_wp[:])
            raT = wpool.tile([128, N], BF16, tag="raT")
            rbT = wpool.tile([128, N], BF16, tag="rbT")
            nc.sync.dma_start(out=raT[:], in_=i_raq[:])
            nc.sync.dma_start(out=rbT[:], in_=i_rbq[:])
            mask01 = wpool.tile([128, 2], BF16, tag="mask01")
            nc.vector.memset(mask01[:], 0.0)
            nc.vector.memset(mask01[0:64, 0:1], 1.0)
            nc.vector.memset(mask01[64:128, 1:2], 1.0)
            eps_sb = wpool.tile([128, 1], F32, tag="eps")
            nc.vector.memset(eps_sb[:], EPS)
            if with_qkv_bias:
                qkbT = wpool.tile([128, 8], F32, tag="qkbT")
                nc.sync.dma_start(out=qkbT[:], in_=i_qkbT[:])
                vb_sb = wpool.tile([1, FL], BF16, tag="vb")
                nc.sync.dma_start(out=vb_sb[:], in_=i_vb[:])
                ones1 = wpool.tile([1, 128], BF16, tag="ones1")
                nc.sync.dma_start(out=ones1[:], in_=i_ones1[:])
            if with_proj_bias:
                pb_sb = wpool.tile([128, KC], F32, tag="pb")
                nc.sync.dma_start(out=pb_sb[:], in_=i_pb[:])

            qT = pp.tile([128, 4, N], BF16, tag="qT")     # [f%128, fc, n]
            kT = pp.tile([128, 4, N], BF16, tag="kT")
            yT = pp.tile([128, 4, N], BF16, tag="yT")
            vball = pp.tile([128, NCHUNK, HL * 96], BF16, tag="vball")
            # per-head 96-col block: [1 | zeros(31) | v(64)].  Even head h
            # slices [96h+32, 96h+160): oT rows 0-63, sums row 64.  Odd head h
            # slices [96h-32, 96h+96): oT rows 64-127, sums row 32.  Unused
            # psum rows collect garbage and are never read.
            vview = vball[:].rearrange("p jc (h c) -> p jc h c", c=96)
            nc.vector.memset(vview[:, :, :, 0:32], 0.0)
            nc.vector.memset(vview[:, :, :, 0:1], 1.0)

            xT_sb = wpool.tile([128, NT, KC, 512], BF16, tag="xT")
            # split the first n-block per kc so the first GEMM can start
            # after 128KB instead of 1MB
            for kc in range(KC):
                nc.sync.dma_start(out=xT_sb[:, 0, kc], in_=i_xT[:, 0, kc])
            for nt in range(1, NT):
                nc.sync.dma_start(out=xT_sb[:, nt], in_=i_xT[:, nt])

            # ======== Phase 1a: q/k in transposed layout + rms + rope ======
            # The GEMM writes straight into qT/kT; stats ride a mask-matmul;
            # rstd (Ln/Exp + DMA-bounce broadcast) and the in-place rope are
            # deferred by 1-2 iterations so the DMA round trip and the ACT/
            # DVE queues never stall the GEMM pipeline.
            pend_stat = []
            pend_rstd = []
            pend_rope = []
            ROPE_DEFER = 2 if shared_tables else 1

            def flush_stats():
                while pend_stat:
                    pend_stat.pop(0)()

            def make_rstd_block(ti, fc, st):
                def emit():
                    lns = scr.tile([2, N], BF16, tag="lns", bufs=1,
                                   name=f"lns{ti}_{fc}")
                    for h2 in range(2):
                        nc.scalar.activation(lns[:, h2 * 1024:(h2 + 1) * 1024],
                                             st[h2][:], AF.Ln,
                                             scale=1.0 / D, bias=eps_sb[0:2])
                    rstd = scr.tile([2, N], BF16, tag="rstd", bufs=1,
                                    name=f"rstd{ti}_{fc}")
                    nc.scalar.activation(rstd[:], lns[:], AF.Exp, scale=-0.5)
                    nc.sync.dma_start(out=d_rstd[ti, fc], in_=rstd[:])
                return emit

            def make_rope_block(ti, fc, dstT):
                def emit():
                    for nh in range(2):
                        nsl = slice(nh * 1024, (nh + 1) * 1024)
                        rbcast = scr.tile([128, 1024], BF16, tag="rbcast",
                                          name=f"rbc{ti}_{fc}_{nh}")
                        nc.sync.dma_start(
                            out=rbcast[:],
                            in_=bass.AP(tensor=d_rstd,
                                        offset=(ti * 4 + fc) * 2 * N + nh * 1024,
                                        ap=[[N, 2], [0, 64], [1, 1024]]))
                        nc.vector.tensor_mul(dstT[:, fc, nsl], dstT[:, fc, nsl],
                                             rbcast[:])  # xn in place
                        t1 = scr.tile([128, 1024], BF16, tag="t1",
                                      name=f"t1{ti}_{fc}_{nh}")
                        nc.vector.tensor_mul(t1[:], dstT[:, fc, nsl],
                                             raT[:, nsl])
                        u = scr.tile([128, 1024], BF16, tag="ush",
                                     name=f"u{ti}_{fc}_{nh}")
                        nc.vector.tensor_mul(u[:], dstT[:, fc, nsl],
                                             rbT[:, nsl])
                        sh = scr.tile([128, 1024], BF16, tag="ush",
                                      name=f"sh{ti}_{fc}_{nh}")
                        nc.vector.stream_shuffle(sh[:], u[:], PAIRSWAP)
                        nc.gpsimd.tensor_add(dstT[:, fc, nsl], t1[:], sh[:])
                return emit

            for it in range(8):
                ti, fc = divmod(it, 4)
                nm, dstT = (("q", qT), ("k", kT))[ti]
                # order matters: finish (i-1)'s last stats matmul, then its
                # Ln/Exp (which free the avA/avB stats psum for this
                # iteration), then the 2-back rope block
                flush_stats()
                if pend_rstd:
                    pend_rstd.pop(0)()
                if it == 4 and not shared_tables:
                    while pend_rope:
                        pend_rope.pop(0)()
                    nc.sync.dma_start(out=raT[:], in_=i_rak[:])
                    nc.sync.dma_start(out=rbT[:], in_=i_rbk[:])
                if len(pend_rope) > ROPE_DEFER:
                    pend_rope.pop(0)()
                fsl = slice(fc * 128, (fc + 1) * 128)
                st = {}
                for h2 in range(2):
                    st[h2] = psp.tile([2, 1024], F32, tag=("avA", "avB")[h2],
                                      name=f"st{ti}_{fc}_{h2}")
                for nt in range(NT):
                    nsl = slice(nt * 512, (nt + 1) * 512)
                    ps = psp.tile([128, 512], F32, tag=("sc0", "sc1")[nt % 2],
                                  name=f"qk{ti}_{fc}_{nt}")
                    for kc in range(KC):
                        nc.tensor.matmul(ps[:], w_sb[nm][:, kc, fsl],
                                         xT_sb[:, nt, kc, :],
                                         start=(kc == 0), stop=(kc == KC - 1))
                    if with_qkv_bias:
                        nc.scalar.activation(
                            dstT[:, fc, nsl], ps[:], AF.Identity,
                            bias=qkbT[:, ti * 4 + fc:ti * 4 + fc + 1])
                    else:
                        nc.scalar.copy(out=dstT[:, fc, nsl], in_=ps[:])
                    sq = scr.tile([128, 512], BF16, tag="t1",
                                  name=f"sq{ti}_{fc}_{nt}")
                    nc.vector.tensor_mul(sq[:], dstT[:, fc, nsl],
                                         dstT[:, fc, nsl])
                    flush_stats()
                    pend_stat.append(
                        (lambda st_=st[nt // 2], sq_=sq, c_=(nt % 2) * 512:
                         nc.tensor.matmul(st_[:, c_:c_ + 512], mask01[:],
                                          sq_[:], start=True, stop=True)))
                pend_rstd.append(make_rstd_block(ti, fc, st))
                pend_rope.append(make_rope_block(ti, fc, dstT))
            flush_stats()
            while pend_rstd:
                pend_rstd.pop(0)()

            # ======== Phase 1b: v in natural layout into vball ============
            for m in range(NCHUNK):
                ps = psp.tile([128, 512], F32, tag=("sc0", "sc1")[m % 2],
                              name=f"v{m}")
                if with_qkv_bias:
                    nc.tensor.matmul(ps[:], ones1[:], vb_sb[:],
                                     start=True, stop=False)
                for kc in range(KC):
                    nc.tensor.matmul(ps[:], xT_sb[:, m // 4, kc,
                                                  (m % 4) * 128:(m % 4 + 1) * 128],
                                     w_sb["v"][:, kc, :],
                                     start=(kc == 0 and not with_qkv_bias),
                                     stop=(kc == KC - 1))
                nc.scalar.copy(out=vview[:, m, :, 32:96],
                               in_=ps[:].rearrange("p (h d) -> p h d", h=HL))
                if m == 1:
                    # last q/k rope blocks run while the v GEMMs keep PE busy
                    while pend_rope:
                        pend_rope.pop(0)()

            # ================= Phase 2: attention per head-pair =============
            PHASES = int(os.environ.get("KERNEL_PHASES", "3"))
            for fc in range(4 if PHASES >= 2 else 0):
                for ih in range(IH):
                    isl = slice(ih * 1024, (ih + 1) * 1024)
                    av = {0: psp.tile([128, 1024], F32, tag="avA", name=f"avA{fc}_{ih}"),
                          1: psp.tile([128, 1024], F32, tag="avB", name=f"avB{fc}_{ih}")}
                    # per jc: scores (both halves) -> exps -> AVs of the
                    # previous jc (one-step skew keeps PE off ACT's critical
                    # path; pt bufs cover the extra lifetime)
                    pt_t = {}

                    def do_av(jc):
                        for half in range(2):
                            h = 2 * fc + half
                            vs = 96 * h + 32 if half == 0 else 96 * h - 32
                            pt = pt_t.pop((jc, half))
                            for i2 in range(2):
                                nc.tensor.matmul(
                                    av[half][:, i2 * 512:(i2 + 1) * 512],
                                    vball[:, jc, vs:vs + 128],
                                    pt[:, i2 * 512:(i2 + 1) * 512],
                                    start=(jc == 0), stop=(jc == NCHUNK - 1))

                    for jc in range(NCHUNK):
                        jsl = slice(jc * 128, (jc + 1) * 128)
                        scs = []
                        for half in range(2):
                            po = 64 * half
                            sc = psp.tile([128, 1024], F32, tag=f"sc{half}",
                                          name=f"sc{fc}_{ih}_{jc}_{half}")
                            lhs = kT[po:po + 64, fc, jsl]
                            for i2 in range(2):
                                nc.tensor.matmul(
                                    sc[:, i2 * 512:(i2 + 1) * 512], lhs,
                                    qT[po:po + 64, fc,
                                       ih * 1024 + i2 * 512:
                                       ih * 1024 + (i2 + 1) * 512],
                                    start=True, stop=True)
                            scs.append(sc)
                        if os.environ.get("KERNEL_NOSM"):
                            continue
                        for half in range(2):
                            pt = ptp.tile([128, 1024], BF16, tag=f"pt{half}",
                                          name=f"pt{fc}_{ih}_{jc}_{half}")
                            nc.scalar.activation(pt[:], scs[half][:], AF.Exp,
                                                 scale=float(SCALE))
                            pt_t[(jc, half)] = pt
                        if jc > 0:
                            do_av(jc - 1)
                    if not os.environ.get("KERNEL_NOSM"):
                        do_av(NCHUNK - 1)
                    # normalize: yT[f, n] = av_oT[f, n] * (1 / av_sums[n]).
                    # even head: oT rows 0-63, sums row 64; odd head: oT rows
                    # 64-127, sums row 32 (lane-aligned by vball layout).
                    # Copy BOTH halves' AV psum -> sbuf first (frees all 4 av
                    # psum banks for the next i-half ASAP), then run the
                    # reciprocal (fast DVE approx) + gpsimd partition
                    # broadcast + final mult off the psum critical path.
                    if not os.environ.get("KERNEL_NONORM"):
                        avs_t = {}
                        for half in range(2):
                            avs = rlp.tile([128, 1024], F32, tag="avs",
                                           name=f"avs{fc}_{ih}_{half}")
                            if half == 0:
                                nc.vector.tensor_copy(out=avs[0:65, :],
                                                      in_=av[half][0:65, :])
                            else:
                                nc.vector.tensor_copy(out=avs[32:33, :],
                                                      in_=av[half][32:33, :])
                                nc.vector.tensor_copy(out=avs[64:128, :],
                                                      in_=av[half][64:128, :])
                            avs_t[half] = avs
                        for half in range(2):
                            po = 64 * half
                            srow = 64 if half == 0 else 32
                            avs = avs_t.pop(half)
                            # reciprocal_approx_fast and partition_broadcast
                            # both require base-partition-0 inputs; stage the
                            # sums row at partition 0 first.
                            s_0 = rlp.tile([1, 1024], F32, tag="s0", bufs=1,
                                           name=f"s0_{fc}_{ih}_{half}")
                            nc.vector.tensor_copy(out=s_0[:],
                                                  in_=avs[srow:srow + 1, :])
                            r_l = rlp.tile([1, 1024], F32, tag="rl", bufs=1,
                                           name=f"rl{fc}_{ih}_{half}")
                            nc.vector.reciprocal_approx_fast(
                                out=r_l[:], in_=s_0[:])
                            r_h = rlp.tile([1, 1024], BF16, tag="rlh", bufs=1,
                                           name=f"rlh{fc}_{ih}_{half}")
                            nc.vector.tensor_copy(out=r_h[:], in_=r_l[:])
                            rbc = rbp.tile([128, 1024], BF16, tag="rbc",
                                           name=f"rbc{fc}_{ih}_{half}")
                            nc.gpsimd.partition_broadcast(
                                rbc[:, :], r_h[:], channels=128)
                            nc.vector.tensor_mul(yT[po:po + 64, fc, isl],
                                                 avs[po:po + 64, :],
                                                 rbc[po:po + 64, :])

            # ================= Phase 3: output projection ===================
            for cc in range(KC if PHASES >= 3 else 0):
                for nt in range(4):
                    fp = psp.tile([128, 512], F32,
                                  tag=("sc0", "sc1", "avA", "avB")[(cc * 4 + nt) % 4],
                                  name=f"fp{cc}_{nt}")
                    for kc in range(4):
                        nc.tensor.matmul(
                            fp[:], wp_sb[:, kc, cc * 128:(cc + 1) * 128],
                            yT[:, kc, nt * 512:(nt + 1) * 512],
                            start=(kc == 0), stop=(kc == 3))
                    so = outp.tile([128, 512], BF16, tag="so")
                    if with_proj_bias:
                        nc.scalar.activation(so[:], fp[:], AF.Identity,
                                             bias=pb_sb[:, cc:cc + 1])
                    elif (cc * 4 + nt) % 2 == 0:
                        nc.scalar.copy(out=so[:], in_=fp[:])
                    else:
                        nc.vector.tensor_copy(out=so[:], in_=fp[:])
                    # SWDGE casting DMA (bf16 -> f32) keeps the staging tile
                    # small; gpsimd is idle here
                    nc.gpsimd.dma_start(
                        out=o_FT[cc * 128:(cc + 1) * 128,
                                 nt * 512:(nt + 1) * 512],
                        in_=so[:])

    nc.compile()
    return nc


def _host_prep(x, qkv_w, qkv_b, proj_w, proj_b, qn_w, kn_w, rope_cos, rope_sin):
    """Build the 8 per-core input maps."""
    x = np.asarray(x, dtype=np.float32)
    qkv_w = np.asarray(qkv_w, dtype=np.float32)
    qkv_b = np.asarray(qkv_b, dtype=np.float32)
    proj_w = np.asarray(proj_w, dtype=np.float32)
    proj_b = np.asarray(proj_b, dtype=np.float32)
    qn_w = np.asarray(qn_w, dtype=np.float32)
    kn_w = np.asarray(kn_w, dtype=np.float32)
    rope_cos = np.asarray(rope_cos, dtype=np.float32)
    rope_sin = np.asarray(rope_sin, dtype=np.float32)

    # f-major rope tables, rows permuted by DMAP (pair-adjacent), with the
    # rms-norm weight folded in:
    #   out[p] = xn[p]*a[d(p)] + xn[p^1]*b[d(p)]
    # so the u-table row p must carry b[d(p^1)] (it lands at p^1 post-swap).
    def tables(w):
        a = rope_cos * w[None, :]                    # [N, D]
        b = np.empty_like(rope_sin)
        b[:, 0:32] = -rope_sin[:, 0:32] * w[None, 32:64]
        b[:, 32:64] = rope_sin[:, 32:64] * w[None, 0:32]
        aT = a[:, DMAP].T                            # [64, N]
        pair = DMAP[np.arange(64) ^ 1]
        bT = b[:, pair].T                            # [64, N]
        aT = np.concatenate([aT, aT], axis=0)        # [128, N]
        bT = np.concatenate([bT, bT], axis=0)
        return (np.ascontiguousarray(aT).astype(ml_dtypes.bfloat16),
                np.ascontiguousarray(bT).astype(ml_dtypes.bfloat16))

    raq, rbq = tables(qn_w)
    shared_tables = bool(np.array_equal(qn_w, kn_w))
    if not shared_tables:
        rak, rbk = tables(kn_w)
    with_qkv_bias = bool(np.any(qkv_b))
    with_proj_bias = bool(np.any(proj_b))
    pb = np.ascontiguousarray(proj_b.reshape(KC, 128).T)

    in_maps = []
    for ci in range(8):
        b, hh = divmod(ci, 2)
        fsl = slice(hh * FL, hh * FL + FL)
        xb = np.ascontiguousarray(
            x[b].T.reshape(KC, 128, NT, 512).transpose(1, 2, 0, 3)
        ).astype(ml_dtypes.bfloat16)                 # [128, NT, KC, 512]
        m = {
            "xT": xb,
            # q/k weight columns permuted so GEMM output row p holds dim
            # d(p) (matches the permuted rope tables); scores are invariant
            # because q and k use the same permutation.  All weights are
            # pre-tiled to [128(c), KC, F] so the device DMA is contiguous.
            "wq": np.ascontiguousarray(
                qkv_w[fsl, :].T[:, FPERM].reshape(KC, 128, FL)
                .transpose(1, 0, 2)).astype(ml_dtypes.bfloat16),
            "wk": np.ascontiguousarray(
                qkv_w[C:][fsl, :].T[:, FPERM].reshape(KC, 128, FL)
                .transpose(1, 0, 2)).astype(ml_dtypes.bfloat16),
            "wv": np.ascontiguousarray(
                qkv_w[2 * C:][fsl, :].T.reshape(KC, 128, FL)
                .transpose(1, 0, 2)).astype(ml_dtypes.bfloat16),
            "wp": np.ascontiguousarray(
                proj_w[:, fsl].T.reshape(4, 128, C)
                .transpose(1, 0, 2)).astype(ml_dtypes.bfloat16),
            "raq": raq, "rbq": rbq,
        }
        if not shared_tables:
            m["rak"] = rak
            m["rbk"] = rbk
        if with_qkv_bias:
            # per-partition bias for q/k (transposed layout, permuted rows)
            qkbT = np.zeros((128, 8), dtype=np.float32)
            for ti, off in ((0, 0), (1, C)):
                bb = qkv_b[off:][fsl]                # [FL]
                for fc in range(4):
                    blk = bb[fc * 128:(fc + 1) * 128].reshape(2, 64)
                    qkbT[0:64, ti * 4 + fc] = blk[0][DMAP]
                    qkbT[64:128, ti * 4 + fc] = blk[1][DMAP]
            m["qkbT"] = qkbT
            m["vb"] = qkv_b[2 * C:][fsl].reshape(1, FL).astype(ml_dtypes.bfloat16)
            m["ones1"] = np.ones((1, 128), dtype=ml_dtypes.bfloat16)
        if with_proj_bias:
            m["pb"] = pb
        in_maps.append(m)
    return in_maps, (with_qkv_bias, with_proj_bias, shared_tables)


def kernel(x, qkv_w, qkv_b, proj_w, proj_b, qn_w, kn_w, rope_cos, rope_sin,
           _trace=False):
    global _PROGRAM
    in_maps, key = _host_prep(x, qkv_w, qkv_b, proj_w, proj_b, qn_w, kn_w,
                              rope_cos, rope_sin)
    if _PROGRAM is None or _PROGRAM[0] != key:
        _PROGRAM = (key, _build_program(*key))
    nc = _PROGRAM[1]
    kwargs = {}
    if _trace:
        kwargs = dict(trace=True, trace_cores=[0])
    res = run_bass_kernel_spmd(nc, in_maps, core_ids=list(range(8)), **kwargs)
    if _trace:
        kernel.last_exec_ns = res.exec_time_ns
        kernel.last_results = res
    out = np.empty((B, N, C), dtype=np.float32)
    for b in range(B):
        ft = res.results[2 * b]["FT"] + res.results[2 * b + 1]["FT"]
        out[b] = ft.T
    return out
